# revision 18
# baseline (speedup 1.0000x reference)
"""Viterbi CRF decode (B=512, T=1024, L=48) on 8 Trainium2 NeuronCores.

Data-parallel over batch: 64 batches per core. On-core layout packs the
64 batches onto 128 SBUF partitions as (batch, half) pairs p = 2b + h;
partition (b, h) computes the Viterbi recurrence for output tags
j in [24h, 24h+24) and holds the full 48-entry v vector in
"own-half-first" rotated order, so every instruction uses
partition-uniform access patterns.

All compute runs on VectorE (GPSIMD is deliberately idle: it shares an
SBUF port with the DVE and measurably serializes against it). Three
runtime-registered custom DVE ops carry the fused steps:

  VIT_BP  : mq = select(sch == pm_row, -Idx, -FLT_MAX)  (one op replaces
            is_equal + iota-mult); a native row-max reduce then yields
            -(first-occurrence flat index); a tiny constant add converts
            to position-R space (48 - i_pos) and downcasts to bf16 bph.
  VIT_FIX : bpf += M * ((bpf > 24)*-48 + 24) converts position-R values
            from half-swapped source partitions to global tag-R space
            during backtrack chunk prep.
  VIT_BT  : out = select(jm2 == R_{t+1}, bp_t, -FLT_MAX); accum_out =
            max -> paths[t]. One DVE op per backtrack step (also reused
            for the final-tag argmax).

Tie handling matches the reference first-occurrence rule exactly within
a half; across halves the pick order is own-half-first rather than
global-tag order (exact fp32 cross-half max ties are ~never observed).
"""

import sys

for _p in ("/opt/trn_rl_repo",):
    if _p not in sys.path:
        sys.path.insert(0, _p)

import numpy as np

import concourse.bacc as bacc
import concourse.tile as tile
from concourse import mybir
from concourse.bass_utils import run_bass_kernel_spmd

B, T, L = 512, 1024, 48
LH = L // 2  # 24: tags per partition
NCORES = 8
BL = B // NCORES  # 64 batches per core
P = 2 * BL  # 128 partitions
F32 = mybir.dt.float32
BF16 = mybir.dt.bfloat16

FMAX = float(np.finfo(np.float32).max)

_OPS = {}


def _get_ops():
    """Register the custom DVE ops (idempotent; runtime registration)."""
    if _OPS:
        return _OPS
    from concourse import dve_ops as dops
    from concourse.dve_spec import (
        Spec, Src0, Src1, C0, C1, Zero, MaxNeg, Idx, eq, select, maxx,
        lower, _has_src1,
    )
    from concourse.dve_uop import DveOpSpec

    def ref_bp(in0, in1, s0, s1, imm2):
        p_ = in0.shape[0]
        a = in0.reshape(p_, -1).astype(np.float32)
        b = np.broadcast_to(np.asarray(in1, np.float32).reshape(p_, -1),
                            a.shape)
        idx = np.arange(a.shape[1], dtype=np.float32)[None]
        return np.where(a == b, -idx, -FMAX).reshape(in0.shape)

    def ref_bt(in0, in1, s0, s1, imm2):
        p_ = in0.shape[0]
        a = in0.reshape(p_, -1).astype(np.float32)
        b = np.asarray(in1, np.float32).reshape(p_, -1)
        body = np.where(a == np.asarray(s0, np.float32).reshape(p_, 1),
                        b, -FMAX)
        return (body.reshape(in0.shape),
                body.max(axis=1, keepdims=True).astype(np.float32))

    def ref_fix(in0, in1, s0, s1, imm2):
        a = in0.astype(np.float32)
        m = np.broadcast_to(np.asarray(in1, np.float32).reshape(
            in0.shape[0], -1), a.reshape(in0.shape[0], -1).shape
        ).reshape(a.shape)
        return a + m * ((a > s0) * s1 + s0)

    specs = [
        ("VIT_BP", Spec(body=select(eq(Src0, Src1), Zero - Idx, MaxNeg),
                        reference=ref_bp)),
        ("VIT_BT", Spec(body=select(eq(Src0, C0), Src1, MaxNeg),
                        accum=maxx, reference=ref_bt)),
        ("VIT_FIX", Spec(body=Src0 + Src1 * ((Src0 > C0) * C1 + C0),
                         reference=ref_fix)),
    ]
    for name, spec in specs:
        ex = next((o for o in dops.OPS if o.name == name), None)
        if ex is None:
            opcode = dops._CUSTOM_DVE_ROW_BASE + len(dops.OPS)
            shas = {}
            for ver in ("v3", "v4"):
                uops = lower(spec, ver=ver)
                shas[ver] = DveOpSpec(name=name, opcode=opcode, uops=uops,
                                      rd1_en=_has_src1(spec)).sha(ver)
            ex = dops.DveOp(name, spec, subdim=False, uops_sha=shas)
            dops.OPS.append(ex)
            dops.CUSTOM_DVE_SPECS[name] = spec
            dops._SUB_OPCODE_FOR_NAME[name] = opcode
        _OPS[name] = ex
    return _OPS


def build_program(bl=BL, t_len=T, debug=False, kb=8, we=64, wb=64,
                  skip_bp=False, skip_bt=False, gp_dummy=0, a_split=0,
                  eq_bufs=1, sch_bufs=2, **_ignored):
    """Per-core Bass program. kb: bp-extraction batch depth. gp_dummy:
    issue an independent GPSIMD tensor op of this many elems/step (port-
    contention probe). a_split: rows of the forward add done on GPSIMD."""
    ops = _get_ops()
    p = 2 * bl
    nc = bacc.Bacc("TRN2", target_bir_lowering=False, debug=debug)

    emis = nc.dram_tensor("emis", [p, t_len, LH], F32, kind="ExternalInput")
    v0 = nc.dram_tensor("v0", [p, L], F32, kind="ExternalInput")
    transt4 = nc.dram_tensor("transt4", [p, LH, L], F32, kind="ExternalInput")
    iotarev = nc.dram_tensor("iotarev", [p, L], F32, kind="ExternalInput")
    jm2 = nc.dram_tensor("jm2", [p, L], F32, kind="ExternalInput")
    endrep = nc.dram_tensor("endrep", [p, L], F32, kind="ExternalInput")
    corr = nc.dram_tensor("corr", [p, kb * LH], F32, kind="ExternalInput")
    mfix = nc.dram_tensor("mfix", [p, L], F32, kind="ExternalInput")
    paths_out = nc.dram_tensor("paths", [p, t_len], mybir.dt.int32,
                               kind="ExternalOutput")

    we = min(we, t_len)
    wb = min(wb, t_len)  # backtrack chunk width
    swap = [(i ^ 1) for i in range(32)]

    with tile.TileContext(nc) as tc:
        with (
            tc.tile_pool(name="consts", bufs=1) as consts,
            tc.tile_pool(name="hist", bufs=1) as hist,
            tc.tile_pool(name="echunks", bufs=2) as echunks,
            tc.tile_pool(name="sch", bufs=sch_bufs) as schpool,
            tc.tile_pool(name="eqp", bufs=eq_bufs) as eqpool,
            tc.tile_pool(name="work", bufs=2) as work,
            tc.tile_pool(name="vf", bufs=2) as vfpool,
            tc.tile_pool(name="bt", bufs=2) as btpool,
        ):
            tt4 = consts.tile([p, LH, L], F32)
            nc.sync.dma_start(out=tt4, in_=transt4.ap())
            ior = consts.tile([p, L], F32)
            nc.sync.dma_start(out=ior, in_=iotarev.ap())
            jm2t = consts.tile([p, L], F32)
            nc.sync.dma_start(out=jm2t, in_=jm2.ap())
            endt = consts.tile([p, L], F32)
            nc.sync.dma_start(out=endt, in_=endrep.ap())
            corrt = consts.tile([p, kb * LH], F32)
            nc.sync.dma_start(out=corrt, in_=corr.ap())
            mfixt = consts.tile([p, L], F32)
            nc.sync.dma_start(out=mfixt, in_=mfix.ap())

            bph = hist.tile([p, t_len - 1, LH], BF16)  # bp hist, position-R
            paths = hist.tile([p, t_len], F32)  # global tag-R

            vcur = vfpool.tile([p, L], F32, tag="vf")
            nc.sync.dma_start(out=vcur, in_=v0.ap())

            if gp_dummy:
                gda = consts.tile([p, gp_dummy], F32)
                nc.vector.memset(gda, 1.0)

            # ---------------- forward ----------------
            def flush_bp(sch, pmh, kn, t0):
                """Extract bp for steps t0..t0+kn-1 (bph rows t0-1..)."""
                mq = eqpool.tile([p, kb, LH, L], F32, tag="mq")
                tmpr = eqpool.tile([p, kb * LH], F32, tag="tmpr")
                mq3 = mq[:, :kn].rearrange("p k j i -> p (k j) i")
                sch3 = sch[:, :kn].rearrange("p k j i -> p (k j) i")
                pm_b3 = (pmh[:, :kn, :].rearrange("p k j -> p (k j)")
                         .unsqueeze(2).broadcast_to([p, kn * LH, L]))
                nc.vector._custom_dve(ops["VIT_BP"], out=mq3, in0=sch3,
                                      in1=pm_b3)
                nc.vector.tensor_reduce(out=tmpr[:, : kn * LH], in_=mq3,
                                        axis=mybir.AxisListType.X,
                                        op=mybir.AluOpType.max)
                bslice = (bph[:, t0 - 1 : t0 - 1 + kn, :]
                          .rearrange("p k j -> p (k j)"))
                nc.vector.tensor_tensor(out=bslice, in0=tmpr[:, : kn * LH],
                                        in1=corrt[:, : kn * LH],
                                        op=mybir.AluOpType.add)

            e_tile = None
            sch = pmh = None
            t0 = 1
            for t in range(1, t_len):
                if (t - 1) % we == 0:
                    t1 = min(t + we, t_len)
                    e_tile = echunks.tile([p, we, LH], F32, tag="e")
                    nc.sync.dma_start(out=e_tile[:, : t1 - t, :],
                                      in_=emis.ap()[:, t:t1, :])
                k = (t - 1) % kb
                if k == 0:
                    t0 = t
                    sch = schpool.tile([p, kb, LH, L], F32, tag="sch")
                    pmh = schpool.tile([p, kb, LH], F32, tag="pmh")
                if gp_dummy:
                    gdo = eqpool.tile([p, gp_dummy], F32, tag="gdo")
                    nc.gpsimd.tensor_mul(out=gdo, in0=gda, in1=gda)
                if a_split > 0:
                    v_b1 = (vcur[:, :].unsqueeze(1)
                            .broadcast_to([p, a_split, L]))
                    nc.gpsimd.tensor_add(out=sch[:, k, 0:a_split, :],
                                         in0=v_b1, in1=tt4[:, 0:a_split, :])
                    v_b2 = (vcur[:, :].unsqueeze(1)
                            .broadcast_to([p, LH - a_split, L]))
                    nc.vector.tensor_add(out=sch[:, k, a_split:LH, :],
                                         in0=v_b2, in1=tt4[:, a_split:LH, :])
                else:
                    v_b = vcur[:, :].unsqueeze(1).broadcast_to([p, LH, L])
                    nc.vector.tensor_add(out=sch[:, k], in0=v_b, in1=tt4)
                nc.vector.tensor_reduce(out=pmh[:, k, :], in_=sch[:, k],
                                        axis=mybir.AxisListType.X,
                                        op=mybir.AluOpType.max)
                vnext = vfpool.tile([p, L], F32, tag="vf")
                nc.vector.tensor_add(out=vnext[:, 0:LH], in0=pmh[:, k, :],
                                     in1=e_tile[:, (t - 1) % we, :])
                nc.vector.stream_shuffle(out=vnext[:, LH:L],
                                         in_=vnext[:, 0:LH], mask=swap)
                vcur = vnext
                if (k == kb - 1 or t == t_len - 1) and not skip_bp:
                    flush_bp(sch, pmh, k + 1, t0)

            # ---------------- final tag ----------------
            vfin = work.tile([p, L], F32, tag="vfin")
            nc.vector.tensor_add(out=vfin, in0=vcur, in1=endt)
            mfin = work.tile([p, 1], F32, tag="mfin")
            nc.vector.tensor_reduce(out=mfin, in_=vfin,
                                    axis=mybir.AxisListType.X,
                                    op=mybir.AluOpType.max)
            scr0 = work.tile([p, L], F32, tag="scr")
            nc.vector._custom_dve(ops["VIT_BT"], out=scr0, in0=vfin,
                                  in1=ior, s0=mfin,
                                  accum_out=paths[:, t_len - 1 : t_len])

            # ---------------- backtrack ----------------
            nchunks = 0 if (skip_bt or skip_bp) else (t_len - 1 + wb - 1) // wb
            for c in range(nchunks - 1, -1, -1):
                c0 = c * wb
                c1 = min(c0 + wb, t_len - 1)
                wn = c1 - c0
                bpf = btpool.tile([p, wb, 2, LH], BF16, tag="bpf")
                nc.vector.tensor_copy(out=bpf[:, :wn, 0, :],
                                      in_=bph[:, c0:c1, :])
                nc.vector.stream_shuffle(out=bpf[:, :wn, 1, :],
                                         in_=bph[:, c0:c1, :], mask=swap)
                bpf2 = bpf[:, :wn].rearrange("p w c j -> p w (c j)")
                m_b = (mfixt[:, :].unsqueeze(1).broadcast_to([p, wn, L]))
                nc.vector._custom_dve(ops["VIT_FIX"], out=bpf2, in0=bpf2,
                                      in1=m_b, s0=float(LH), s1=-float(L))
                for t in range(c1 - 1, c0 - 1, -1):
                    scr = work.tile([p, L], BF16, tag="scr")
                    bps = bpf[:, t - c0].rearrange("p c j -> p (c j)")
                    nc.vector._custom_dve(ops["VIT_BT"], out=scr, in0=jm2t,
                                          in1=bps, s0=paths[:, t + 1 : t + 2],
                                          accum_out=paths[:, t : t + 1])

            # ---------------- output: tag = 48 - R, cast int32 ----------
            tagi = hist.tile([p, t_len], mybir.dt.int32)
            nc.vector.tensor_scalar(out=tagi, in0=paths, scalar1=-1.0,
                                    scalar2=float(L),
                                    op0=mybir.AluOpType.mult,
                                    op1=mybir.AluOpType.add)
            nc.sync.dma_start(out=paths_out.ap(), in_=tagi)

    nc.compile()
    return nc


def make_core_inputs(emissions, transitions, start_transitions,
                     end_transitions, bl=BL, t_len=T, ncores=NCORES, kb=8):
    """Host-side prep: per-core input dicts (numpy, all fp32)."""
    p = 2 * bl
    harr = np.arange(p) % 2
    barr = np.arange(p) // 2
    gi = (np.arange(L)[None, :] + LH * harr[:, None]) % L  # [p, L]
    gj = LH * harr[:, None] + np.arange(LH)[None, :]  # [p, LH]
    tt4 = transitions[gi[:, None, :], gj[:, :, None]].astype(np.float32)
    iotarev = (L - gi).astype(np.float32)
    k = np.arange(L)[None, :]
    j_of = np.where(k < LH, LH * harr[:, None] + k,
                    LH * (1 - harr[:, None]) + (k - LH))
    jm2 = (L - j_of).astype(np.float32)
    endrep = end_transitions[gi].astype(np.float32)
    # flush row-correction: bph = reduce_max(mq) + 48*row + 48
    row = np.arange(kb * LH, dtype=np.float32)
    corr = np.broadcast_to(L * row + L, (p, kb * LH)).astype(np.float32)
    # backtrack fixup mask: 1.0 where the source half hs = h XOR c is 1
    cidx = (k >= LH).astype(np.int64)  # slot c for flat (c,j) position
    mfixv = ((harr[:, None] ^ cidx) == 1).astype(np.float32)

    in_maps = []
    for c in range(ncores):
        em = emissions[c * bl : (c + 1) * bl, :t_len]  # [bl, t, L]
        e_pre = np.ascontiguousarray(
            em.reshape(bl, t_len, 2, LH).transpose(0, 2, 1, 3)
            .reshape(p, t_len, LH))
        vfull = (start_transitions[None, :] + em[:, 0]).astype(np.float32)
        v0 = vfull[barr[:, None], gi]
        in_maps.append({
            "emis": e_pre,
            "v0": np.ascontiguousarray(v0),
            "transt4": tt4,
            "iotarev": iotarev,
            "jm2": jm2,
            "endrep": endrep,
            "corr": np.ascontiguousarray(corr),
            "mfix": np.ascontiguousarray(mfixv),
        })
    return in_maps


_prog_cache = {}
_run_opts = {"trace": False}
_last_result = None


def kernel(emissions, mask, transitions, start_transitions, end_transitions):
    global _last_result
    emissions = np.asarray(emissions, dtype=np.float32)
    transitions = np.asarray(transitions, dtype=np.float32)
    start_transitions = np.asarray(start_transitions, dtype=np.float32)
    end_transitions = np.asarray(end_transitions, dtype=np.float32)

    key = (BL, T)
    if key not in _prog_cache:
        _prog_cache[key] = build_program()
    nc = _prog_cache[key]

    in_maps = make_core_inputs(emissions, transitions, start_transitions,
                               end_transitions)
    res = run_bass_kernel_spmd(nc, in_maps, core_ids=list(range(NCORES)),
                               trace=_run_opts["trace"])
    _last_result = res
    outs = [r["paths"][::2, :] for r in res.results]  # h=0 partitions
    return np.concatenate(outs, axis=0).astype(np.int32)


if __name__ == "__main__":
    pass
